# revision 6
# baseline (speedup 1.0000x reference)
"""Batched merged linear (LoRA-style) Trainium2 Bass kernel — fp8/bf16 hybrid.

Problem: x:[16,1024,4096] f32, weight:[4096,4096], bias:[4096],
lora_A:[8,32,4096], lora_B:[8,2048,16].
out[m] = x[m] @ W.T + bias, rank-16 LoRA correction (scale 2.0) on output
columns [0:1024] and [2048:3072].

Strategy: one adapter per NeuronCore (8 cores), LoRA merged into the weight
on the host (W_eff[m] = W + scatter(2 * B_m @ A_m)).

Precision-hybrid matmul: the contraction K=4096 is split. K-planes
[0, K8) run as fp8e4 (e4m3) matmuls in DoubleRow perf mode — 2 K-planes
per PE cycle, 2x bf16 throughput, measured exactly 216 ns per 256-plane
DR instruction on HW. Planes [K8, 4096) run in bf16. With K8=1024 the
per-tile instruction count drops 32 -> 24 bf16 + 4 DR (0.875x), and the
quantization error (measured against the fp32 reference on the real
inputs) is 1.90e-2 < 2e-2 tolerance.

Scale handling: fp8 operands are pre-scaled (x*2^2, W*2^8) so products
carry 2^10; the bf16 weight is pre-scaled by 2^10 to match; one PSUM
accumulates both, and the activation eviction applies scale 2^-10 and
the bias in fp32. Output is written bf16 (negligible vs budget).

Loop: token-chunk (t4) outer, output-chunk (oc) inner; x for a chunk is
SBUF-resident, weights stream per tile (w8 then wb) on the sync ring.
The ramp is fed fp8-first (4x less bytes per K-plane) and x is split
across the scalar+gpsimd rings so the PE starts within ~4 us.
"""

import sys
import types

import numpy as np
import ml_dtypes

# run_bass_kernel_spmd imports antenv.axon_hooks when tracing is requested;
# the module is absent on this image. Register a None-hook stub so a stray
# BASS_TRACE=1 degrades to "no trace" instead of ImportError.
try:
    import antenv
    import antenv.axon_hooks  # noqa: F401
except ImportError:
    if "antenv" in sys.modules:
        _m = types.ModuleType("antenv.axon_hooks")
        _m._hook = None
        _m.set_axon_ntff_profile_hook = lambda h: setattr(_m, "_hook", h)
        _m.get_axon_ntff_profile_hook = lambda: _m._hook
        sys.modules["antenv.axon_hooks"] = _m
        sys.modules["antenv"].axon_hooks = _m

BF16 = ml_dtypes.bfloat16
E4 = ml_dtypes.float8_e4m3  # TRN FP8_EXP4 (max +-240) == IEEE e4m3
M_ADAPT, G, R, BLOCK = 8, 2, 16, 1024
SCALING = 2.0
D = 4096           # in_features == out_features
T = 2048           # tokens per core (2 batches x 1024)
K8 = 1024          # fp8 K-planes (multiple of 256)
J = K8 // 256      # DR instructions per tile
KB = (D - K8) // 128   # bf16 K chunks of 128
OC = 32            # output chunks of 128
T4 = 4             # token chunks of 512
TN = 512           # matmul moving free dim
SX = 4.0           # fp8 x scale (2^2)
SW = 256.0         # fp8 W scale (2^8)
SOUT = 2.0 ** -10  # psum descale (1/(SX*SW))

_CACHE = {}


def _build_bass():
    import concourse.mybir as mybir
    import concourse.tile as tile
    from concourse import bacc

    DR = mybir.MatmulPerfMode.DoubleRow
    nc = bacc.Bacc("TRN2", target_bir_lowering=False, debug=False, num_devices=8)

    x8d = nc.dram_tensor("x8T", [128, T4, J, 2, TN], mybir.dt.float8e4,
                         kind="ExternalInput").ap()
    xbd = nc.dram_tensor("xbT", [128, T4, KB, TN], mybir.dt.bfloat16,
                         kind="ExternalInput").ap()
    w8d = nc.dram_tensor("w8T", [128, OC, J, 2, 128], mybir.dt.float8e4,
                         kind="ExternalInput").ap()
    wbd = nc.dram_tensor("wbT", [128, OC, KB, 128], mybir.dt.bfloat16,
                         kind="ExternalInput").ap()
    biasd = nc.dram_tensor("bias2", [128, OC], mybir.dt.float32,
                           kind="ExternalInput").ap()
    od = nc.dram_tensor("outT", [128, OC, T], mybir.dt.bfloat16,
                        kind="ExternalOutput").ap()

    with tile.TileContext(nc) as tc:
        with tc.tile_pool(name="x8p", bufs=2) as x8p, \
             tc.tile_pool(name="xbp", bufs=2) as xbp, \
             tc.tile_pool(name="cst", bufs=1) as cst, \
             tc.tile_pool(name="w8p", bufs=4) as w8p, \
             tc.tile_pool(name="wbp", bufs=4) as wbp, \
             tc.tile_pool(name="op", bufs=4) as op, \
             tc.tile_pool(name="pp", bufs=6, space="PSUM") as pp, \
             tc.tile_pool(name="ph", bufs=2, space="PSUM") as ph:

            bias_s = cst.tile([128, OC], mybir.dt.float32)
            nc.scalar.dma_start(bias_s[:], biasd)

            # x chunks: fp8 part first (small, feeds the DR matmuls early),
            # bf16 part split in sub-DMAs across scalar+gpsimd rings so the
            # ramp is fed at ~2x one ring's bandwidth.
            x_tiles = {}

            def emit_x(t4, ramp=False):
                x8_s = x8p.tile([128, J, 2, TN], mybir.dt.float8e4,
                                tag="x8", name=f"x8_{t4}")
                if ramp:
                    # per-j sub-DMAs (128KB each) so the very first DR
                    # matmul starts as soon as j0 lands
                    for j in range(J):
                        nc.scalar.dma_start(x8_s[:, j], x8d[:, t4, j])
                else:
                    nc.scalar.dma_start(x8_s[:], x8d[:, t4])
                subs = []
                if ramp:
                    bounds = [(0, 2), (2, 4), (4, 8), (8, 12),
                              (12, 16), (16, 20), (20, 24)]
                else:
                    bounds = [(0, 8), (8, 16), (16, 24)]
                for s, (k0, k1) in enumerate(bounds):
                    st = xbp.tile([128, k1 - k0, TN], mybir.dt.bfloat16,
                                  tag=f"xb{s}", name=f"xb_{t4}_{s}")
                    eng = nc.gpsimd if (ramp and s % 2 == 1) else nc.scalar
                    eng.dma_start(st[:], xbd[:, t4, k0:k1])
                    subs.extend([(st, k0)] * (k1 - k0))
                x_tiles[t4] = (x8_s, subs)

            emit_x(0, ramp=True)
            for t4 in range(T4):
                x8_s, xb_s = x_tiles[t4]
                for oc in range(OC):
                    # prefetch next token chunk mid-pass, spread across ocs
                    if oc == 16 and t4 + 1 < T4:
                        emit_x(t4 + 1)
                    # weights: w8 first (first matmuls of the tile), then wb.
                    w8_s = w8p.tile([128, J, 2, 128], mybir.dt.float8e4,
                                    tag="w8", name=f"w8_{t4}_{oc}")
                    nc.sync.dma_start(w8_s[:], w8d[:, oc])
                    if t4 == 0 and oc < 2:
                        # ramp: finer weight sub-DMAs so early matmuls start
                        # as soon as their slice lands
                        wb_sub = [wbp.tile([128, 8, 128], mybir.dt.bfloat16,
                                           tag=f"wr{s}", name=f"wb_{oc}_{s}")
                                  for s in range(3)]
                        for s in range(3):
                            nc.sync.dma_start(wb_sub[s][:],
                                              wbd[:, oc, 8 * s:8 * (s + 1)])
                        wb_of = lambda ko, wb_sub=wb_sub: \
                            wb_sub[ko // 8][:, ko % 8, :]
                    else:
                        wb_s = wbp.tile([128, KB, 128], mybir.dt.bfloat16,
                                        tag="wb", name=f"wb_{t4}_{oc}")
                        nc.sync.dma_start(wb_s[:], wbd[:, oc])
                        wb_of = lambda ko, wb_s=wb_s: wb_s[:, ko, :]

                    # Final two blocks: two 256-wide chains so the first
                    # half's evict+DMA overlaps the second half's matmuls,
                    # and outputs ride the idle sync ring for a short drain.
                    # Interleave: [DR j][6x bf16] x4. Back-to-back DR matmuls
                    # expose ~214ns of 256-col LDWEIGHTS per tile (the weight
                    # buffer pipeline needs >=1 matmul of lead time); spacing
                    # the DR matmuls gives every LDWEIGHTS a full window.
                    GRP = KB // J  # bf16 matmuls between DR matmuls
                    if t4 == T4 - 1 and oc >= 30:
                        for h in range(2):
                            hs = slice(h * (TN // 2), (h + 1) * (TN // 2))
                            pt = ph.tile([128, TN // 2], mybir.dt.float32,
                                         tag="pph", name=f"pp_{t4}_{oc}_{h}")
                            for j in range(J):
                                nc.tensor.matmul(
                                    pt[:], w8_s[:, j], x8_s[:, j, :, hs],
                                    start=(j == 0), stop=False, perf_mode=DR)
                                for ko in range(GRP * j, GRP * (j + 1)):
                                    st, k0 = xb_s[ko]
                                    nc.tensor.matmul(
                                        pt[:], wb_of(ko), st[:, ko - k0, hs],
                                        start=False, stop=(ko == KB - 1))
                            o_s = op.tile([128, TN // 2], mybir.dt.bfloat16,
                                          tag="oh", name=f"o_{t4}_{oc}_{h}")
                            nc.scalar.activation(
                                o_s[:], pt[:],
                                mybir.ActivationFunctionType.Identity,
                                bias=bias_s[:, oc:oc + 1], scale=SOUT)
                            nc.sync.dma_start(
                                od[:, oc,
                                   t4 * TN + h * (TN // 2):
                                   t4 * TN + (h + 1) * (TN // 2)], o_s[:])
                        continue

                    pt = pp.tile([128, TN], mybir.dt.float32, tag="pp",
                                 name=f"pp_{t4}_{oc}")
                    for j in range(J):
                        nc.tensor.matmul(pt[:], w8_s[:, j], x8_s[:, j],
                                         start=(j == 0), stop=False,
                                         perf_mode=DR)
                        for ko in range(GRP * j, GRP * (j + 1)):
                            st, k0 = xb_s[ko]
                            nc.tensor.matmul(pt[:], wb_of(ko),
                                             st[:, ko - k0, :],
                                             start=False, stop=(ko == KB - 1))
                    o_s = op.tile([128, TN], mybir.dt.bfloat16, tag="o",
                                  name=f"o_{t4}_{oc}")
                    nc.scalar.activation(
                        o_s[:], pt[:], mybir.ActivationFunctionType.Identity,
                        bias=bias_s[:, oc:oc + 1], scale=SOUT)
                    nc.gpsimd.dma_start(od[:, oc, t4 * TN:(t4 + 1) * TN],
                                        o_s[:])

    nc.compile()
    return nc


def _get_nc():
    if "nc" not in _CACHE:
        _CACHE["nc"] = _build_bass()
    return _CACHE["nc"]


def _host_prep(x, weight, bias, lora_A, lora_B):
    bias2 = np.ascontiguousarray(bias.reshape(OC, 128).T.astype(np.float32))
    in_maps = []
    for c in range(M_ADAPT):
        x_m = x[2 * c:2 * c + 2].reshape(T, D).astype(np.float32)
        # x8T[p, t4, j, s, n] = e4m3(4 * x[t4*512+n, 256j+2p+s])
        x8 = np.ascontiguousarray(
            (x_m[:, :K8] * SX).reshape(T4, TN, J, 128, 2)
            .transpose(3, 0, 2, 4, 1)).astype(E4)
        # xbT[p, t4, ko, n] = bf16(x[t4*512+n, K8+128ko+p])
        xb = np.ascontiguousarray(
            x_m[:, K8:].reshape(T4, TN, KB, 128)
            .transpose(3, 0, 2, 1)).astype(BF16)
        # merge LoRA into the weight: W_eff = W + scatter(2 * B_g @ A_g)
        w_eff = weight.astype(np.float32).copy()
        A = lora_A[c].reshape(G, R, D)
        B = lora_B[c].reshape(G, BLOCK, R)
        w_eff[0:1024] += SCALING * (B[0] @ A[0])
        w_eff[2048:3072] += SCALING * (B[1] @ A[1])
        # w8T[p, oc, j, s, m] = e4m3(256 * W[128oc+m, 256j+2p+s])
        w8 = np.ascontiguousarray(
            (w_eff[:, :K8] * SW).reshape(OC, 128, J, 128, 2)
            .transpose(3, 0, 2, 4, 1)).astype(E4)
        # wbT[p, oc, ko, m] = bf16(1024 * W[128oc+m, K8+128ko+p])
        wb = np.ascontiguousarray(
            (w_eff[:, K8:] * (SX * SW)).reshape(OC, 128, KB, 128)
            .transpose(3, 0, 2, 1)).astype(BF16)
        in_maps.append({"x8T": x8, "xbT": xb, "w8T": w8, "wbT": wb,
                        "bias2": bias2})
    return in_maps


def run(inputs, trace=False):
    """Build (cached), run on 8 cores, return (output, BassKernelResults)."""
    from concourse import bass_utils
    nc = _get_nc()
    in_maps = _host_prep(inputs["x"], inputs["weight"], inputs["bias"],
                         inputs["lora_A"], inputs["lora_B"])
    res = bass_utils.run_bass_kernel_spmd(
        nc, in_maps, core_ids=list(range(8)), trace=trace)
    out = np.empty((16, 1024, D), np.float32)
    for c in range(M_ADAPT):
        out_m = res.results[c]["outT"].astype(np.float32) \
            .transpose(2, 1, 0).reshape(T, D)
        out[2 * c] = out_m[:1024]
        out[2 * c + 1] = out_m[1024:]
    return out, res


def kernel(x, weight, bias, lora_A, lora_B):
    out, _ = run({"x": np.asarray(x), "weight": np.asarray(weight),
                  "bias": np.asarray(bias), "lora_A": np.asarray(lora_A),
                  "lora_B": np.asarray(lora_B)})
    return out


# revision 10
# speedup vs baseline: 1.0130x; 1.0130x over previous
"""Batched merged linear (LoRA-style) Trainium2 Bass kernel — fp8/bf16 hybrid.

Problem: x:[16,1024,4096] f32, weight:[4096,4096], bias:[4096],
lora_A:[8,32,4096], lora_B:[8,2048,16].
out[m] = x[m] @ W.T + bias, rank-16 LoRA correction (scale 2.0) on output
columns [0:1024] and [2048:3072].

Strategy: one adapter per NeuronCore (8 cores), LoRA merged into the weight
on the host (W_eff[m] = W + scatter(2 * B_m @ A_m)).

Precision-hybrid matmul: the contraction K=4096 is split. K-planes
[0, K8) run as fp8e4 (e4m3) matmuls in DoubleRow perf mode — 2 K-planes
per PE cycle, 2x bf16 throughput, measured exactly 216 ns per 256-plane
DR instruction on HW. Planes [K8, 4096) run in bf16. With K8=1024 the
per-tile instruction count drops 32 -> 24 bf16 + 4 DR (0.875x), and the
quantization error (measured against the fp32 reference on the real
inputs) is 1.90e-2 < 2e-2 tolerance.

Scale handling: fp8 operands are pre-scaled (x*2^2, W*2^8) so products
carry 2^10; the bf16 weight is pre-scaled by 2^10 to match; one PSUM
accumulates both, and the activation eviction applies scale 2^-10 and
the bias in fp32. Output is written bf16 (negligible vs budget).

Loop: token-chunk (t4) outer, output-chunk (oc) inner; x for a chunk is
SBUF-resident, weights stream per tile (w8 then wb) on the sync ring.
The ramp is fed fp8-first (4x less bytes per K-plane) and x is split
across the scalar+gpsimd rings so the PE starts within ~4 us.
"""

import sys
import types

import numpy as np
import ml_dtypes

# run_bass_kernel_spmd imports antenv.axon_hooks when tracing is requested;
# the module is absent on this image. Register a None-hook stub so a stray
# BASS_TRACE=1 degrades to "no trace" instead of ImportError.
try:
    import antenv
    import antenv.axon_hooks  # noqa: F401
except ImportError:
    if "antenv" in sys.modules:
        _m = types.ModuleType("antenv.axon_hooks")
        _m._hook = None
        _m.set_axon_ntff_profile_hook = lambda h: setattr(_m, "_hook", h)
        _m.get_axon_ntff_profile_hook = lambda: _m._hook
        sys.modules["antenv.axon_hooks"] = _m
        sys.modules["antenv"].axon_hooks = _m

BF16 = ml_dtypes.bfloat16
E4 = ml_dtypes.float8_e4m3  # TRN FP8_EXP4 (max +-240) == IEEE e4m3
M_ADAPT, G, R, BLOCK = 8, 2, 16, 1024
SCALING = 2.0
D = 4096           # in_features == out_features
T = 2048           # tokens per core (2 batches x 1024)
K8 = 1024          # fp8 K-planes (multiple of 256)
J = K8 // 256      # DR instructions per tile
KB = (D - K8) // 128   # bf16 K chunks of 128
OC = 32            # output chunks of 128
T4 = 4             # token chunks of 512
TN = 512           # matmul moving free dim
SX = 4.0           # fp8 x scale (2^2)
SW = 256.0         # fp8 W scale (2^8)
SOUT = 2.0 ** -10  # psum descale (1/(SX*SW))

_CACHE = {}


def _build_bass():
    import concourse.mybir as mybir
    import concourse.tile as tile
    from concourse import bacc

    DR = mybir.MatmulPerfMode.DoubleRow
    nc = bacc.Bacc("TRN2", target_bir_lowering=False, debug=False, num_devices=8)

    x8d = nc.dram_tensor("x8T", [128, T4, J, 2, TN], mybir.dt.float8e4,
                         kind="ExternalInput").ap()
    xbd = nc.dram_tensor("xbT", [128, T4, KB, TN], mybir.dt.bfloat16,
                         kind="ExternalInput").ap()
    w8d = nc.dram_tensor("w8T", [128, OC, J, 2, 128], mybir.dt.float8e4,
                         kind="ExternalInput").ap()
    wbd = nc.dram_tensor("wbT", [128, OC, KB, 128], mybir.dt.bfloat16,
                         kind="ExternalInput").ap()
    biasd = nc.dram_tensor("bias2", [128, OC], mybir.dt.float32,
                           kind="ExternalInput").ap()
    od = nc.dram_tensor("outT", [128, OC, T], mybir.dt.bfloat16,
                        kind="ExternalOutput").ap()

    with tile.TileContext(nc) as tc:
        with tc.tile_pool(name="x8p", bufs=2) as x8p, \
             tc.tile_pool(name="xbp", bufs=2) as xbp, \
             tc.tile_pool(name="cst", bufs=1) as cst, \
             tc.tile_pool(name="w8p", bufs=8) as w8p, \
             tc.tile_pool(name="wbp", bufs=6) as wbp, \
             tc.tile_pool(name="op", bufs=4) as op, \
             tc.tile_pool(name="pp", bufs=4, space="PSUM") as pp, \
             tc.tile_pool(name="ph", bufs=4, space="PSUM") as ph:

            bias_s = cst.tile([128, OC], mybir.dt.float32)
            nc.scalar.dma_start(bias_s[:], biasd)

            # x chunks: fp8 part first (small, feeds the DR matmuls early),
            # bf16 part split in sub-DMAs across scalar+gpsimd rings so the
            # ramp is fed at ~2x one ring's bandwidth.
            x_tiles = {}

            def emit_x(t4, ramp=False):
                x8_s = x8p.tile([128, J, 2, TN], mybir.dt.float8e4,
                                tag="x8", name=f"x8_{t4}")
                if ramp:
                    # per-j sub-DMAs (128KB each) so the very first DR
                    # matmul starts as soon as j0 lands
                    for j in range(J):
                        nc.scalar.dma_start(x8_s[:, j], x8d[:, t4, j])
                else:
                    nc.scalar.dma_start(x8_s[:], x8d[:, t4])
                subs = []
                if ramp:
                    bounds = [(0, 2), (2, 4), (4, 8), (8, 12),
                              (12, 16), (16, 20), (20, 24)]
                else:
                    bounds = [(0, 8), (8, 16), (16, 24)]
                for s, (k0, k1) in enumerate(bounds):
                    st = xbp.tile([128, k1 - k0, TN], mybir.dt.bfloat16,
                                  tag=f"xb{s}", name=f"xb_{t4}_{s}")
                    eng = nc.gpsimd if (ramp and s % 2 == 1) else nc.scalar
                    eng.dma_start(st[:], xbd[:, t4, k0:k1])
                    subs.extend([(st, k0)] * (k1 - k0))
                x_tiles[t4] = (x8_s, subs)

            emit_x(0, ramp=True)
            # Tiles run in batches of BW output-chunks: the batch's DR (fp8)
            # chains run back-to-back first (pure DR sustains 216ns/instr;
            # each bf16<->DR transition costs ~300ns, so batching pays it
            # once per BW tiles instead of per tile), then each tile's bf16
            # chain + eviction. The final batch splits its last two tiles
            # into 256-wide halves so evict+DMA overlaps matmuls on the
            # drain, riding the idle sync ring.
            BW = 4
            for t4 in range(T4):
                x8_s, xb_s = x_tiles[t4]
                for ocg in range(0, OC, BW):
                    if ocg == 16 and t4 + 1 < T4:
                        emit_x(t4 + 1)
                    ocs = list(range(ocg, ocg + BW))
                    tail = (t4 == T4 - 1 and ocg == OC - BW)
                    full_ocs = ocs[:-2] if tail else ocs
                    w8_ss = {}
                    for oc in ocs:
                        w8_s = w8p.tile([128, J, 2, 128], mybir.dt.float8e4,
                                        tag="w8", name=f"w8_{t4}_{oc}")
                        nc.sync.dma_start(w8_s[:], w8d[:, oc])
                        w8_ss[oc] = w8_s
                    wb_ofs = {}
                    for oc in ocs:
                        if t4 == 0 and oc < 2:
                            # ramp: finer weight sub-DMAs so early matmuls
                            # start as soon as their slice lands
                            wb_sub = [wbp.tile([128, 8, 128],
                                               mybir.dt.bfloat16,
                                               tag=f"wr{s}",
                                               name=f"wb_{oc}_{s}")
                                      for s in range(3)]
                            for s in range(3):
                                nc.sync.dma_start(
                                    wb_sub[s][:],
                                    wbd[:, oc, 8 * s:8 * (s + 1)])
                            wb_ofs[oc] = lambda ko, wb_sub=wb_sub: \
                                wb_sub[ko // 8][:, ko % 8, :]
                        else:
                            wb_s = wbp.tile([128, KB, 128],
                                            mybir.dt.bfloat16,
                                            tag="wb", name=f"wb_{t4}_{oc}")
                            nc.sync.dma_start(wb_s[:], wbd[:, oc])
                            wb_ofs[oc] = lambda ko, wb_s=wb_s: wb_s[:, ko, :]

                    # ---- DR phase: one fp8 chain across the batch ----
                    pts = {}
                    for oc in full_ocs:
                        pt = pp.tile([128, TN], mybir.dt.float32, tag="pp",
                                     name=f"pp_{t4}_{oc}")
                        for j in range(J):
                            nc.tensor.matmul(pt[:], w8_ss[oc][:, j],
                                             x8_s[:, j], start=(j == 0),
                                             stop=False, perf_mode=DR)
                        pts[oc] = pt
                    if tail:
                        for oc in ocs[-2:]:
                            for h in range(2):
                                hs = slice(h * (TN // 2), (h + 1) * (TN // 2))
                                pt = ph.tile([128, TN // 2],
                                             mybir.dt.float32, tag="pph",
                                             name=f"pp_{t4}_{oc}_{h}")
                                for j in range(J):
                                    nc.tensor.matmul(
                                        pt[:], w8_ss[oc][:, j],
                                        x8_s[:, j, :, hs],
                                        start=(j == 0), stop=False,
                                        perf_mode=DR)
                                pts[(oc, h)] = pt

                    # ---- bf16 phase + evictions, per tile ----
                    for oc in full_ocs:
                        pt = pts[oc]
                        for ko in range(KB):
                            st, k0 = xb_s[ko]
                            nc.tensor.matmul(pt[:], wb_ofs[oc](ko),
                                             st[:, ko - k0, :],
                                             start=False,
                                             stop=(ko == KB - 1))
                        o_s = op.tile([128, TN], mybir.dt.bfloat16, tag="o",
                                      name=f"o_{t4}_{oc}")
                        nc.scalar.activation(
                            o_s[:], pt[:],
                            mybir.ActivationFunctionType.Identity,
                            bias=bias_s[:, oc:oc + 1], scale=SOUT)
                        nc.gpsimd.dma_start(
                            od[:, oc, t4 * TN:(t4 + 1) * TN], o_s[:])
                    if tail:
                        for oc in ocs[-2:]:
                            for h in range(2):
                                hs = slice(h * (TN // 2), (h + 1) * (TN // 2))
                                pt = pts[(oc, h)]
                                for ko in range(KB):
                                    st, k0 = xb_s[ko]
                                    nc.tensor.matmul(
                                        pt[:], wb_ofs[oc](ko),
                                        st[:, ko - k0, hs],
                                        start=False, stop=(ko == KB - 1))
                                o_s = op.tile([128, TN // 2],
                                              mybir.dt.bfloat16, tag="oh",
                                              name=f"o_{t4}_{oc}_{h}")
                                nc.scalar.activation(
                                    o_s[:], pt[:],
                                    mybir.ActivationFunctionType.Identity,
                                    bias=bias_s[:, oc:oc + 1], scale=SOUT)
                                nc.sync.dma_start(
                                    od[:, oc,
                                       t4 * TN + h * (TN // 2):
                                       t4 * TN + (h + 1) * (TN // 2)],
                                    o_s[:])

    nc.compile()
    return nc


def _get_nc():
    if "nc" not in _CACHE:
        _CACHE["nc"] = _build_bass()
    return _CACHE["nc"]


def _host_prep(x, weight, bias, lora_A, lora_B):
    bias2 = np.ascontiguousarray(bias.reshape(OC, 128).T.astype(np.float32))
    in_maps = []
    for c in range(M_ADAPT):
        x_m = x[2 * c:2 * c + 2].reshape(T, D).astype(np.float32)
        # x8T[p, t4, j, s, n] = e4m3(4 * x[t4*512+n, 256j+2p+s])
        x8 = np.ascontiguousarray(
            (x_m[:, :K8] * SX).reshape(T4, TN, J, 128, 2)
            .transpose(3, 0, 2, 4, 1)).astype(E4)
        # xbT[p, t4, ko, n] = bf16(x[t4*512+n, K8+128ko+p])
        xb = np.ascontiguousarray(
            x_m[:, K8:].reshape(T4, TN, KB, 128)
            .transpose(3, 0, 2, 1)).astype(BF16)
        # merge LoRA into the weight: W_eff = W + scatter(2 * B_g @ A_g)
        w_eff = weight.astype(np.float32).copy()
        A = lora_A[c].reshape(G, R, D)
        B = lora_B[c].reshape(G, BLOCK, R)
        w_eff[0:1024] += SCALING * (B[0] @ A[0])
        w_eff[2048:3072] += SCALING * (B[1] @ A[1])
        # w8T[p, oc, j, s, m] = e4m3(256 * W[128oc+m, 256j+2p+s])
        w8 = np.ascontiguousarray(
            (w_eff[:, :K8] * SW).reshape(OC, 128, J, 128, 2)
            .transpose(3, 0, 2, 4, 1)).astype(E4)
        # wbT[p, oc, ko, m] = bf16(1024 * W[128oc+m, K8+128ko+p])
        wb = np.ascontiguousarray(
            (w_eff[:, K8:] * (SX * SW)).reshape(OC, 128, KB, 128)
            .transpose(3, 0, 2, 1)).astype(BF16)
        in_maps.append({"x8T": x8, "xbT": xb, "w8T": w8, "wbT": wb,
                        "bias2": bias2})
    return in_maps


def run(inputs, trace=False):
    """Build (cached), run on 8 cores, return (output, BassKernelResults)."""
    from concourse import bass_utils
    nc = _get_nc()
    in_maps = _host_prep(inputs["x"], inputs["weight"], inputs["bias"],
                         inputs["lora_A"], inputs["lora_B"])
    res = bass_utils.run_bass_kernel_spmd(
        nc, in_maps, core_ids=list(range(8)), trace=trace)
    out = np.empty((16, 1024, D), np.float32)
    for c in range(M_ADAPT):
        out_m = res.results[c]["outT"].astype(np.float32) \
            .transpose(2, 1, 0).reshape(T, D)
        out[2 * c] = out_m[:1024]
        out[2 * c + 1] = out_m[1024:]
    return out, res


def kernel(x, weight, bias, lora_A, lora_B):
    out, _ = run({"x": np.asarray(x), "weight": np.asarray(weight),
                  "bias": np.asarray(bias), "lora_A": np.asarray(lora_A),
                  "lora_B": np.asarray(lora_B)})
    return out
